# revision 17
# baseline (speedup 1.0000x reference)
"""Multi-head attention Trainium2 kernel (8 NeuronCores, SPMD).

Sharding: core c handles batch b=c//2, query-row half r=c%2 (1024 q rows),
all 8 heads, full key range.

Per-core pipeline (S-orientation scores, fused softmax):
  QT = (8*Wq^T) x_q, KT = Wk^T x_k  (fp32r), V = x_v Wv^T (bf16, with a
  ones column appended per head so PV also accumulates Z = row-sums).
  per (head, 128-row q tile):
    S = QT_h^T KT_h                  (PSUM fp32, q on partitions)
    one fused DVE pass: msc = -(S + maskbias), nmx = min(msc) = -rowmax
    P = exp(-msc + nmx)              (ACT, bf16; masked entries -> 0)
    PT strips via DMA crossbar transpose (sbuf->sbuf, 16x128 xbar tiles)
  per head: OT[65,1024] = V'^T PT accumulated over k (row 64 = Z), then
  1/Z broadcast (Pool partition_broadcast) is folded into the scramble
  copy, and the reference's head-scrambled reshape is folded into the
  output projection as 4 K=128 matmuls against pre-scrambled Wo chunks.
Query rows are processed in j-major permuted order (q = 8t+j <-> p =
t+128j) so every access stays contiguous; the host permutes query/mask
rows and reassembles the output.
"""

import os
import numpy as np
import ml_dtypes

KSTAGE = int(os.environ.get("KSTAGE", "9"))

import concourse.bass as bass
import concourse.mybir as mybir
from concourse import bacc
from concourse.bass_utils import run_bass_kernel_spmd
from concourse.tile import TileContext
from concourse.masks import make_identity

F32 = mybir.dt.float32
F32R = mybir.dt.float32r
BF16 = mybir.dt.bfloat16
AF = mybir.ActivationFunctionType
ALU = mybir.AluOpType

B, S, E, H, DK = 4, 2048, 512, 8, 64
SQ = S // 2          # q rows per core
NE = E // 128        # 4 embed chunks
NKT = S // 128       # 16 key tiles
NQT = SQ // 128      # 8 q tiles per core
NEG = -1.0e9
N_CORES = 8

BF = ml_dtypes.bfloat16


def build_nc():
    nc = bacc.Bacc(None, target_bir_lowering=False)

    xqT = nc.declare_dram_parameter("xqT", [E, SQ], F32R, isOutput=False)
    xkT = nc.declare_dram_parameter("xkT", [E, S], F32R, isOutput=False)
    xvT = nc.declare_dram_parameter("xvT", [E, S], BF16, isOutput=False)
    mb = nc.declare_dram_parameter("mb", [SQ, S], BF16, isOutput=False)
    wqT = nc.declare_dram_parameter("wqT", [E, E], F32R, isOutput=False)
    wkT = nc.declare_dram_parameter("wkT", [E, E], F32R, isOutput=False)
    wvT = nc.declare_dram_parameter("wvT", [E, E], BF16, isOutput=False)
    woc = nc.declare_dram_parameter("woc", [E, E], BF16, isOutput=False)
    bqt = nc.declare_dram_parameter("bqt", [128, NE], F32, isOutput=False)
    bkt = nc.declare_dram_parameter("bkt", [128, NE], F32, isOutput=False)
    bvr = nc.declare_dram_parameter("bvr", [128, E], F32, isOutput=False)
    bor = nc.declare_dram_parameter("bor", [128, E], F32, isOutput=False)
    out = nc.declare_dram_parameter("out", [SQ, E], F32, isOutput=True)

    with TileContext(nc) as tc:
        with (
            tc.tile_pool(name="const", bufs=1) as constp,
            tc.tile_pool(name="mbp", bufs=1) as mbp,
            tc.tile_pool(name="qk", bufs=1) as qk,
            tc.tile_pool(name="vp", bufs=1) as vp,
        ):
            bq_sb = constp.tile([128, NE], F32, tag="bq", name="bq")
            bk_sb = constp.tile([128, NE], F32, tag="bk", name="bk")
            bv_sb = constp.tile([128, E], F32, tag="bv", name="bv")
            bo_sb = constp.tile([128, E], F32, tag="bo", name="bo")
            nc.sync.dma_start(out=bq_sb[:, :], in_=bqt[:, :])
            nc.sync.dma_start(out=bk_sb[:, :], in_=bkt[:, :])
            nc.sync.dma_start(out=bv_sb[:, :], in_=bvr[:, :])
            nc.sync.dma_start(out=bo_sb[:, :], in_=bor[:, :])
            woc_sb = [constp.tile([128, E], BF16, tag=f"woc{c}", name=f"woc{c}")
                      for c in range(4)]
            for c in range(4):
                nc.sync.dma_start(out=woc_sb[c][:, :],
                                  in_=woc[128 * c:128 * c + 128, :])

            mb_sb = [mbp.tile([128, S], BF16, tag=f"mb{j}", name=f"mb{j}")
                     for j in range(NQT)]
            for j in range(NQT):
                nc.sync.dma_start(out=mb_sb[j][:, :],
                                  in_=mb[128 * j:128 * j + 128, :])

            qt_sb = [qk.tile([128, SQ], F32R, tag=f"qt{m}", name=f"qt{m}")
                     for m in range(NE)]
            kt_sb = [qk.tile([128, S], F32R, tag=f"kt{m}", name=f"kt{m}")
                     for m in range(NE)]
            v_sb = [vp.tile([128, 8 * 65], BF16, tag=f"v{k}", name=f"v{k}")
                    for k in range(NKT)]
            for k in range(NKT):
                ones_ap = v_sb[k].rearrange("p (h d) -> p h d", h=H)[:, :, 64:65]
                nc.gpsimd.memset(ones_ap, 1.0)

            # ---------------- projection phase ----------------
            with (
                tc.tile_pool(name="wts", bufs=1) as wts,
                tc.tile_pool(name="xqp", bufs=1) as xqp,
                tc.tile_pool(name="xkp", bufs=2) as xkp,
                tc.tile_pool(name="xvp", bufs=2) as xvp,
                tc.tile_pool(name="pps", bufs=2, space="PSUM") as pps,
            ):
                wq_sb = [wts.tile([128, E], F32R, tag=f"wq{c}", name=f"wq{c}")
                         for c in range(NE)]
                wk_sb = [wts.tile([128, E], F32R, tag=f"wk{c}", name=f"wk{c}")
                         for c in range(NE)]
                wv_sb = [wts.tile([128, E], BF16, tag=f"wv{c}", name=f"wv{c}")
                         for c in range(NE)]
                xq_sb = [xqp.tile([128, SQ], F32R, tag=f"xq{c}", name=f"xq{c}")
                         for c in range(NE)]
                for c in range(NE):
                    nc.sync.dma_start(out=wq_sb[c][:, :],
                                      in_=wqT[128 * c:128 * c + 128, :])
                    nc.sync.dma_start(out=wk_sb[c][:, :],
                                      in_=wkT[128 * c:128 * c + 128, :])
                    nc.sync.dma_start(out=wv_sb[c][:, :],
                                      in_=wvT[128 * c:128 * c + 128, :])
                    nc.sync.dma_start(out=xq_sb[c][:, :],
                                      in_=xqT[128 * c:128 * c + 128, :])

                # Q projection: QT[e',q'] chunks
                for m in range(NE):
                    ps = pps.tile([128, 1024], F32, tag="pps", name=f"psq{m}")
                    for h2 in range(2):
                        for c in range(NE):
                            nc.tensor.matmul(
                                ps[:, 512 * h2:512 * h2 + 512],
                                wq_sb[c][:, 128 * m:128 * m + 128],
                                xq_sb[c][:, 512 * h2:512 * h2 + 512],
                                start=(c == 0), stop=(c == NE - 1))
                    nc.scalar.activation(
                        out=qt_sb[m][:, :], in_=ps[:, :], func=AF.Identity,
                        bias=bq_sb[:, m:m + 1], scale=1.0)

                # K projection, x_k staged in halves
                for kh in range(2):
                    xk_sb = [xkp.tile([128, 1024], F32R, tag=f"xk{c}",
                                      name=f"xk{c}_{kh}") for c in range(NE)]
                    for c in range(NE):
                        nc.sync.dma_start(
                            out=xk_sb[c][:, :],
                            in_=xkT[128 * c:128 * c + 128,
                                    1024 * kh:1024 * kh + 1024])
                    for m in range(NE):
                        ps = pps.tile([128, 1024], F32, tag="pps",
                                      name=f"psk{m}_{kh}")
                        for kq in range(2):
                            for c in range(NE):
                                nc.tensor.matmul(
                                    ps[:, 512 * kq:512 * kq + 512],
                                    wk_sb[c][:, 128 * m:128 * m + 128],
                                    xk_sb[c][:, 512 * kq:512 * kq + 512],
                                    start=(c == 0), stop=(c == NE - 1))
                        nc.scalar.activation(
                            out=kt_sb[m][:, 1024 * kh:1024 * kh + 1024],
                            in_=ps[:, :], func=AF.Identity,
                            bias=bk_sb[:, m:m + 1], scale=1.0)

                # V projection, x_v staged in halves; V' = [V | 1] per head
                for vh in range(2):
                    xv_sb = [xvp.tile([128, 1024], BF16, tag=f"xv{c}",
                                      name=f"xv{c}_{vh}") for c in range(NE)]
                    for c in range(NE):
                        nc.sync.dma_start(
                            out=xv_sb[c][:, :],
                            in_=xvT[128 * c:128 * c + 128,
                                    1024 * vh:1024 * vh + 1024])
                    for ppp in range(4):
                        ps = pps.tile([128, 1024], F32, tag="pps",
                                      name=f"psv{ppp}_{vh}")
                        for sub in range(2):
                            kl = 256 * ppp + 128 * sub  # local k col in half
                            for c in range(NE):
                                nc.tensor.matmul(
                                    ps[:, 512 * sub:512 * sub + 512],
                                    xv_sb[c][:, kl:kl + 128],
                                    wv_sb[c][:, :],
                                    start=(c == 0), stop=(c == NE - 1))
                        for sub in range(2):
                            kt = 8 * vh + 2 * ppp + sub
                            vout = v_sb[kt].rearrange(
                                "p (h d) -> p h d", h=H)[:, :, 0:64]
                            vin = ps[:, 512 * sub:512 * sub + 512].rearrange(
                                "p (h d) -> p h d", h=H)
                            bvv = bv_sb.rearrange("p (h d) -> p h d", h=H)
                            nc.vector.tensor_tensor(
                                out=vout, in0=vin[:, :, :], in1=bvv[:, :, :],
                                op=ALU.add)

            # ---------------- attention phase ----------------
            with (
                tc.tile_pool(name="ptp", bufs=1) as ptp,
                tc.tile_pool(name="pp", bufs=2) as pp,
                tc.tile_pool(name="stats", bufs=2) as stats,
                tc.tile_pool(name="rzp", bufs=2) as rzp,
                tc.tile_pool(name="odp", bufs=2) as odp,
                tc.tile_pool(name="idp", bufs=1) as idp,
                tc.tile_pool(name="scp", bufs=5, space="PSUM") as scp,
                tc.tile_pool(name="otp", bufs=1, space="PSUM") as otp,
            ):
                id_bf = idp.tile([128, 128], BF16, tag="id_bf", name="id_bf")
                make_identity(nc, id_bf[:, :])
                ptall = ptp.tile([128, NKT * 1024], BF16, tag="ptall",
                                 name="ptall")
                ptall_r = ptall.rearrange("p (s q) -> p s q", s=NKT)
                if KSTAGE <= 1:
                    # proj debug: dump QT chunk 0
                    dbg = odp.tile([128, E], F32, tag="dbg", name="dbg")
                    nc.vector.tensor_copy(dbg[:, :],
                                          qt_sb[0][:, 0:E].bitcast(F32))
                    nc.sync.dma_start(out=out[0:128, :], in_=dbg[:, :])
                for h in range(H if KSTAGE >= 4 else (0 if KSTAGE <= 1 else 1)):
                    hm, hp = h // 2, 64 * (h % 2)
                    for j in range(NQT):
                        qch = qt_sb[hm][hp:hp + 64, 128 * j:128 * j + 128]
                        nmx = stats.tile([128, 6], F32, tag="nmx",
                                         name=f"nmx{h}_{j}")
                        scq = []
                        for kq in range(4):
                            sc = scp.tile([128, 512], F32, tag="sc",
                                          name=f"sc{h}_{j}_{kq}")
                            scq.append(sc)
                            ksl = slice(512 * kq, 512 * kq + 512)
                            nc.tensor.matmul(
                                sc[:, :], qch, kt_sb[hm][hp:hp + 64, ksl],
                                start=True, stop=False)
                            nc.tensor.matmul(
                                sc[:, :], id_bf[:, :], mb_sb[j][:, ksl],
                                start=False, stop=True)
                            nc.vector.tensor_reduce(
                                out=nmx[:, kq:kq + 1], in_=sc[:, :],
                                axis=mybir.AxisListType.X, op=ALU.max)
                        nc.vector.tensor_reduce(
                            out=nmx[:, 4:5], in_=nmx[:, 0:4],
                            axis=mybir.AxisListType.X, op=ALU.max,
                            negate=True)
                        p_sb = pp.tile([128, S], BF16, tag="p",
                                       name=f"p{h}_{j}")
                        for kq in range(4):
                            nc.scalar.activation(
                                out=p_sb[:, 512 * kq:512 * kq + 512],
                                in_=scq[kq][:, :],
                                func=AF.Exp, bias=nmx[:, 4:5], scale=1.0)
                        if KSTAGE == 2:
                            dbgp = odp.tile([128, E], F32, tag="dbgp",
                                            name=f"dbgp{j}")
                            nc.vector.tensor_copy(dbgp[:, :], p_sb[:, 0:E])
                            nc.sync.dma_start(
                                out=out[128 * j:128 * j + 128, :],
                                in_=dbgp[:, :])
                            continue
                        nc.sync.dma_start_transpose(
                            ptall_r[:, :, 128 * j:128 * j + 128], p_sb[:, :])
                        if KSTAGE == 3:
                            dbgp = odp.tile([128, E], F32, tag="dbgp",
                                            name=f"dbgp{j}")
                            nc.vector.tensor_copy(
                                dbgp[:, :],
                                ptall[:, 1024 * j:1024 * j + E])
                            nc.sync.dma_start(
                                out=out[128 * j:128 * j + 128, :],
                                in_=dbgp[:, :])
                    if KSTAGE <= 3:
                        continue

                    # PV + Z accumulation: OT[65, SQ] in two halves, row 64 = Z
                    oth = [otp.tile([128, 512], F32, tag=f"ot{qh}",
                                    name=f"ot{qh}_{h}") for qh in range(2)]
                    for qh in range(2):
                        for kt in range(NKT):
                            nc.tensor.matmul(
                                oth[qh][0:65, :],
                                v_sb[kt][:, 65 * h:65 * h + 65],
                                ptall[:, 1024 * kt + 512 * qh:
                                      1024 * kt + 512 * qh + 512],
                                start=(kt == 0), stop=(kt == NKT - 1))

                    zrow = stats.tile([1, SQ], F32, tag="zrow",
                                      name=f"zrow{h}")
                    for qh in range(2):
                        nc.vector.tensor_scalar(
                            out=zrow[:, 512 * qh:512 * qh + 512],
                            in0=oth[qh][64:65, :], scalar1=1.0e-30,
                            scalar2=None, op0=ALU.max)
                    rz = stats.tile([1, SQ], F32, tag="rz", name=f"rz{h}")
                    nc.vector.reciprocal(rz[:, :], zrow[:, :])
                    rzbc = rzp.tile([64, SQ], F32, tag="rzbc", name=f"rzbc{h}")
                    nc.gpsimd.partition_broadcast(rzbc[:, :], rz[:, :])

                    # scramble copy with 1/Z folded in:
                    # otd2[64a+d, 128c+t] = OT[d, 128(2c+a)+t] * rz[...]
                    otd2 = odp.tile([128, 512], BF16, tag="otd2",
                                    name=f"otd2{h}")
                    rz_r = rzbc.rearrange("p (j t) -> p j t", j=8)
                    for a in range(2):
                        for ch in range(2):
                            # global block jj=2c+a: c in {2ch, 2ch+1} ->
                            # oth[ch] local blocks {a, a+2}
                            ot_r = oth[ch].rearrange("p (j t) -> p j t", j=4)
                            nc.vector.tensor_tensor(
                                out=otd2[64 * a:64 * a + 64,
                                         256 * ch:256 * ch + 256].rearrange(
                                    "p (c t) -> p c t", c=2),
                                in0=ot_r[0:64, a::2, :],
                                in1=rz_r[0:64,
                                         4 * ch + a:4 * ch + a + 3:2, :],
                                op=ALU.mult)

                    po = otp.tile([128, 512], F32, tag="po", name=f"po{h}")
                    for c in range(4):
                        nc.tensor.matmul(
                            po[:, :], otd2[:, 128 * c:128 * c + 128],
                            woc_sb[c][:, :], start=(c == 0), stop=(c == 3))
                    o_sb = odp.tile([128, E], F32, tag="osb", name=f"osb{h}")
                    nc.vector.tensor_tensor(
                        out=o_sb[:, :], in0=po[:, :], in1=bo_sb[:, :],
                        op=ALU.add)
                    nc.sync.dma_start(out=out[128 * h:128 * h + 128, :],
                                      in_=o_sb[:, :])

    nc.compile()
    return nc


_NC = None
_last_in_maps = None

# j-major query permutation: position p <-> q row 8*(p%128) + p//128
_QPERM = (8 * (np.arange(SQ) % 128) + np.arange(SQ) // 128)

# output-projection row scramble: woc[128c + 64a + d] = WoT row 64*(2c+a)+d
_p = np.arange(E)
_FIDX = 64 * (2 * (_p // 128) + (_p % 128) // 64) + (_p % 64)


def _get_nc():
    global _NC
    if _NC is None:
        _NC = build_nc()
    return _NC


def kernel(query, key_in, value, mask, Wq, bq, Wk, bk, Wv, bv, Wo, bo):
    query = np.asarray(query, np.float32)
    key_in = np.asarray(key_in, np.float32)
    value = np.asarray(value, np.float32)
    mask = np.asarray(mask)
    Wq = np.asarray(Wq, np.float32)
    Wk = np.asarray(Wk, np.float32)
    Wv = np.asarray(Wv, np.float32)
    Wo = np.asarray(Wo, np.float32)

    sdk = np.sqrt(np.float32(DK))
    wqT = np.ascontiguousarray((Wq * sdk).T)
    wkT = np.ascontiguousarray(Wk.T)
    wvT = np.ascontiguousarray(Wv.T).astype(BF)
    woc = np.ascontiguousarray(Wo.T[_FIDX, :]).astype(BF)
    bqt = np.ascontiguousarray(
        (np.asarray(bq, np.float32) * sdk).reshape(NE, 128).T)
    bkt = np.ascontiguousarray(np.asarray(bk, np.float32).reshape(NE, 128).T)
    bvr = np.ascontiguousarray(
        np.tile(np.asarray(bv, np.float32).reshape(1, E), (128, 1)))
    bor = np.ascontiguousarray(
        np.tile(np.asarray(bo, np.float32).reshape(1, E), (128, 1)))

    in_maps = []
    for c in range(N_CORES):
        b, r = c // 2, c % 2
        q0 = SQ * r
        qsel = q0 + _QPERM
        mbias = np.where(mask[b, qsel, :] == 0, np.float32(NEG),
                         np.float32(0.0)).astype(BF)
        in_maps.append({
            "xqT": np.ascontiguousarray(query[b, qsel, :].T),
            "xkT": np.ascontiguousarray(key_in[b].T),
            "xvT": np.ascontiguousarray(value[b].T).astype(BF),
            "mb": mbias,
            "wqT": wqT, "wkT": wkT, "wvT": wvT, "woc": woc,
            "bqt": bqt, "bkt": bkt, "bvr": bvr, "bor": bor,
        })

    nc = _get_nc()
    global _last_in_maps
    _last_in_maps = in_maps
    res = run_bass_kernel_spmd(nc, in_maps, list(range(N_CORES)))

    full = np.empty((B, S, E), np.float32)
    for c in range(N_CORES):
        b, r = c // 2, c % 2
        oc = res.results[c]["out"]
        for h in range(H):
            full[b, 256 * h + 128 * r:256 * h + 128 * r + 128, :] = \
                oc[128 * h:128 * h + 128, :]
    return full


# revision 22
# speedup vs baseline: 1.0914x; 1.0914x over previous
"""Multi-head attention Trainium2 kernel (8 NeuronCores, SPMD).

Sharding: core c handles batch b=c//2, query-row half r=c%2 (1024 q rows),
all 8 heads, full key range.

Per-core pipeline (S-orientation scores, fused softmax):
  QT = (8*Wq^T) x_q, KT = Wk^T x_k  (fp32r), V = x_v Wv^T (bf16, with a
  ones column appended per head so PV also accumulates Z = row-sums).
  per (head, 128-row q tile):
    S = QT_h^T KT_h                  (PSUM fp32, q on partitions)
    one fused DVE pass: msc = -(S + maskbias), nmx = min(msc) = -rowmax
    P = exp(-msc + nmx)              (ACT, bf16; masked entries -> 0)
    PT strips via DMA crossbar transpose (sbuf->sbuf, 16x128 xbar tiles)
  per head: OT[65,1024] = V'^T PT accumulated over k (row 64 = Z), then
  1/Z broadcast (Pool partition_broadcast) is folded into the scramble
  copy, and the reference's head-scrambled reshape is folded into the
  output projection as 4 K=128 matmuls against pre-scrambled Wo chunks.
Query rows are processed in j-major permuted order (q = 8t+j <-> p =
t+128j) so every access stays contiguous; the host permutes query/mask
rows and reassembles the output.
"""

import os
import numpy as np
import ml_dtypes

KSTAGE = int(os.environ.get("KSTAGE", "9"))

import concourse.bass as bass
import concourse.mybir as mybir
from concourse import bacc
from concourse.bass_utils import run_bass_kernel_spmd
from concourse.tile import TileContext
from concourse.masks import make_identity

F32 = mybir.dt.float32
F32R = mybir.dt.float32r
BF16 = mybir.dt.bfloat16
AF = mybir.ActivationFunctionType
ALU = mybir.AluOpType

B, S, E, H, DK = 4, 2048, 512, 8, 64
SQ = S // 2          # q rows per core
NE = E // 128        # 4 embed chunks
NKT = S // 128       # 16 key tiles
NQT = SQ // 128      # 8 q tiles per core
NEG = -1.0e9
N_CORES = 8

BF = ml_dtypes.bfloat16


def build_nc():
    nc = bacc.Bacc(None, target_bir_lowering=False)

    xqT = nc.declare_dram_parameter("xqT", [E, SQ], F32R, isOutput=False)
    xkT = nc.declare_dram_parameter("xkT", [E, S], F32R, isOutput=False)
    xvT = nc.declare_dram_parameter("xvT", [E, S], BF16, isOutput=False)
    mb = nc.declare_dram_parameter("mb", [SQ, S], BF16, isOutput=False)
    wqT = nc.declare_dram_parameter("wqT", [E, E], F32R, isOutput=False)
    wkT = nc.declare_dram_parameter("wkT", [E, E], F32R, isOutput=False)
    wvT = nc.declare_dram_parameter("wvT", [E, E], BF16, isOutput=False)
    woc = nc.declare_dram_parameter("woc", [E, E], BF16, isOutput=False)
    bqt = nc.declare_dram_parameter("bqt", [128, NE], F32, isOutput=False)
    bkt = nc.declare_dram_parameter("bkt", [128, NE], F32, isOutput=False)
    bvr = nc.declare_dram_parameter("bvr", [128, E], F32, isOutput=False)
    bor = nc.declare_dram_parameter("bor", [128, E], F32, isOutput=False)
    out = nc.declare_dram_parameter("out", [SQ, E], F32, isOutput=True)

    with TileContext(nc) as tc:
        with (
            tc.tile_pool(name="const", bufs=1) as constp,
            tc.tile_pool(name="mbp", bufs=1) as mbp,
            tc.tile_pool(name="qk", bufs=1) as qk,
            tc.tile_pool(name="vp", bufs=1) as vp,
        ):
            bq_sb = constp.tile([128, NE], F32, tag="bq", name="bq")
            bk_sb = constp.tile([128, NE], F32, tag="bk", name="bk")
            bv_sb = constp.tile([128, E], F32, tag="bv", name="bv")
            bo_sb = constp.tile([128, E], F32, tag="bo", name="bo")
            nc.sync.dma_start(out=bq_sb[:, :], in_=bqt[:, :])
            nc.sync.dma_start(out=bk_sb[:, :], in_=bkt[:, :])
            nc.sync.dma_start(out=bv_sb[:, :], in_=bvr[:, :])
            nc.sync.dma_start(out=bo_sb[:, :], in_=bor[:, :])
            woc_sb = [constp.tile([128, E], BF16, tag=f"woc{c}", name=f"woc{c}")
                      for c in range(4)]
            for c in range(4):
                nc.sync.dma_start(out=woc_sb[c][:, :],
                                  in_=woc[128 * c:128 * c + 128, :])

            mb_sb = [mbp.tile([128, S], BF16, tag=f"mb{j}", name=f"mb{j}")
                     for j in range(NQT)]
            for j in range(NQT):
                nc.sync.dma_start(out=mb_sb[j][:, :],
                                  in_=mb[128 * j:128 * j + 128, :])

            qt_sb = [qk.tile([128, SQ], F32R, tag=f"qt{m}", name=f"qt{m}")
                     for m in range(NE)]
            kt_sb = [qk.tile([128, S], F32R, tag=f"kt{m}", name=f"kt{m}")
                     for m in range(NE)]
            v_sb = [vp.tile([128, 8 * 65], BF16, tag=f"v{k}", name=f"v{k}")
                    for k in range(NKT)]
            for k in range(NKT):
                ones_ap = v_sb[k].rearrange("p (h d) -> p h d", h=H)[:, :, 64:65]
                nc.gpsimd.memset(ones_ap, 1.0)

            # ---------------- projection phase ----------------
            with (
                tc.tile_pool(name="wts", bufs=1) as wts,
                tc.tile_pool(name="xqp", bufs=1) as xqp,
                tc.tile_pool(name="xkp", bufs=2) as xkp,
                tc.tile_pool(name="xvp", bufs=2) as xvp,
                tc.tile_pool(name="pps", bufs=2, space="PSUM") as pps,
            ):
                wq_sb = [wts.tile([128, E], F32R, tag=f"wq{c}", name=f"wq{c}")
                         for c in range(NE)]
                wk_sb = [wts.tile([128, E], F32R, tag=f"wk{c}", name=f"wk{c}")
                         for c in range(NE)]
                wv_sb = [wts.tile([128, E], BF16, tag=f"wv{c}", name=f"wv{c}")
                         for c in range(NE)]
                xq_sb = [xqp.tile([128, SQ], F32R, tag=f"xq{c}", name=f"xq{c}")
                         for c in range(NE)]
                for c in range(NE):
                    nc.sync.dma_start(out=wq_sb[c][:, :],
                                      in_=wqT[128 * c:128 * c + 128, :])
                    nc.sync.dma_start(out=wk_sb[c][:, :],
                                      in_=wkT[128 * c:128 * c + 128, :])
                    nc.sync.dma_start(out=wv_sb[c][:, :],
                                      in_=wvT[128 * c:128 * c + 128, :])
                    nc.sync.dma_start(out=xq_sb[c][:, :],
                                      in_=xqT[128 * c:128 * c + 128, :])

                # Q projection: QT[e',q'] chunks
                for m in range(NE):
                    ps = pps.tile([128, 1024], F32, tag="pps", name=f"psq{m}")
                    for h2 in range(2):
                        for c in range(NE):
                            nc.tensor.matmul(
                                ps[:, 512 * h2:512 * h2 + 512],
                                wq_sb[c][:, 128 * m:128 * m + 128],
                                xq_sb[c][:, 512 * h2:512 * h2 + 512],
                                start=(c == 0), stop=(c == NE - 1))
                    nc.scalar.activation(
                        out=qt_sb[m][:, :], in_=ps[:, :], func=AF.Identity,
                        bias=bq_sb[:, m:m + 1], scale=1.0)

                # K projection, x_k staged in halves
                for kh in range(2):
                    xk_sb = [xkp.tile([128, 1024], F32R, tag=f"xk{c}",
                                      name=f"xk{c}_{kh}") for c in range(NE)]
                    for c in range(NE):
                        nc.sync.dma_start(
                            out=xk_sb[c][:, :],
                            in_=xkT[128 * c:128 * c + 128,
                                    1024 * kh:1024 * kh + 1024])
                    for m in range(NE):
                        ps = pps.tile([128, 1024], F32, tag="pps",
                                      name=f"psk{m}_{kh}")
                        for kq in range(2):
                            for c in range(NE):
                                nc.tensor.matmul(
                                    ps[:, 512 * kq:512 * kq + 512],
                                    wk_sb[c][:, 128 * m:128 * m + 128],
                                    xk_sb[c][:, 512 * kq:512 * kq + 512],
                                    start=(c == 0), stop=(c == NE - 1))
                        nc.scalar.activation(
                            out=kt_sb[m][:, 1024 * kh:1024 * kh + 1024],
                            in_=ps[:, :], func=AF.Identity,
                            bias=bk_sb[:, m:m + 1], scale=1.0)

                # V projection, x_v staged in halves; V' = [V | 1] per head
                for vh in range(2):
                    xv_sb = [xvp.tile([128, 1024], BF16, tag=f"xv{c}",
                                      name=f"xv{c}_{vh}") for c in range(NE)]
                    for c in range(NE):
                        nc.sync.dma_start(
                            out=xv_sb[c][:, :],
                            in_=xvT[128 * c:128 * c + 128,
                                    1024 * vh:1024 * vh + 1024])
                    for ppp in range(4):
                        ps = pps.tile([128, 1024], F32, tag="pps",
                                      name=f"psv{ppp}_{vh}")
                        for sub in range(2):
                            kl = 256 * ppp + 128 * sub  # local k col in half
                            for c in range(NE):
                                nc.tensor.matmul(
                                    ps[:, 512 * sub:512 * sub + 512],
                                    xv_sb[c][:, kl:kl + 128],
                                    wv_sb[c][:, :],
                                    start=(c == 0), stop=(c == NE - 1))
                        for sub in range(2):
                            kt = 8 * vh + 2 * ppp + sub
                            vout = v_sb[kt].rearrange(
                                "p (h d) -> p h d", h=H)[:, :, 0:64]
                            vin = ps[:, 512 * sub:512 * sub + 512].rearrange(
                                "p (h d) -> p h d", h=H)
                            bvv = bv_sb.rearrange("p (h d) -> p h d", h=H)
                            nc.vector.tensor_tensor(
                                out=vout, in0=vin[:, :, :], in1=bvv[:, :, :],
                                op=ALU.add)

            # ---------------- attention phase ----------------
            with (
                tc.tile_pool(name="ptp", bufs=2) as ptp,
                tc.tile_pool(name="pp", bufs=2) as pp,
                tc.tile_pool(name="stats", bufs=2) as stats,
                tc.tile_pool(name="rzp", bufs=2) as rzp,
                tc.tile_pool(name="odp", bufs=2) as odp,
                tc.tile_pool(name="idp", bufs=1) as idp,
                tc.tile_pool(name="scp", bufs=5, space="PSUM") as scp,
                tc.tile_pool(name="otp", bufs=1, space="PSUM") as otp,
            ):
                id_bf = idp.tile([128, 128], BF16, tag="id_bf", name="id_bf")
                make_identity(nc, id_bf[:, :])
                if KSTAGE <= 1:
                    # proj debug: dump QT chunk 0
                    dbg = odp.tile([128, E], F32, tag="dbg", name="dbg")
                    nc.vector.tensor_copy(dbg[:, :],
                                          qt_sb[0][:, 0:E].bitcast(F32))
                    nc.sync.dma_start(out=out[0:128, :], in_=dbg[:, :])
                for h in range(H if KSTAGE >= 4 else (0 if KSTAGE <= 1 else 1)):
                    hm, hp = h // 2, 64 * (h % 2)
                    ptall = ptp.tile([128, NKT * 1024], BF16, tag="ptall",
                                     name=f"ptall{h}")
                    ptall_r = ptall.rearrange("p (s q) -> p s q", s=NKT)
                    for j in range(NQT):
                        qch = qt_sb[hm][hp:hp + 64, 128 * j:128 * j + 128]
                        nmx = stats.tile([128, 6], F32, tag="nmx",
                                         name=f"nmx{h}_{j}")
                        scq = []
                        for kq in range(4):
                            sc = scp.tile([128, 512], F32, tag="sc",
                                          name=f"sc{h}_{j}_{kq}")
                            scq.append(sc)
                            ksl = slice(512 * kq, 512 * kq + 512)
                            nc.tensor.matmul(
                                sc[:, :], qch, kt_sb[hm][hp:hp + 64, ksl],
                                start=True, stop=False)
                            nc.tensor.matmul(
                                sc[:, :], id_bf[:, :], mb_sb[j][:, ksl],
                                start=False, stop=True)
                            nc.vector.tensor_reduce(
                                out=nmx[:, kq:kq + 1], in_=sc[:, :],
                                axis=mybir.AxisListType.X, op=ALU.max)
                        nc.vector.tensor_reduce(
                            out=nmx[:, 4:5], in_=nmx[:, 0:4],
                            axis=mybir.AxisListType.X, op=ALU.max,
                            negate=True)
                        p_sb = pp.tile([128, S], BF16, tag="p",
                                       name=f"p{h}_{j}")
                        for kq in range(4):
                            nc.scalar.activation(
                                out=p_sb[:, 512 * kq:512 * kq + 512],
                                in_=scq[kq][:, :],
                                func=AF.Exp, bias=nmx[:, 4:5], scale=1.0)
                        if KSTAGE == 2:
                            dbgp = odp.tile([128, E], F32, tag="dbgp",
                                            name=f"dbgp{j}")
                            nc.vector.tensor_copy(dbgp[:, :], p_sb[:, 0:E])
                            nc.sync.dma_start(
                                out=out[128 * j:128 * j + 128, :],
                                in_=dbgp[:, :])
                            continue
                        xeng = nc.sync if j % 2 == 0 else nc.scalar
                        xeng.dma_start_transpose(
                            ptall_r[:, :, 128 * j:128 * j + 128], p_sb[:, :])
                        if KSTAGE == 3:
                            dbgp = odp.tile([128, E], F32, tag="dbgp",
                                            name=f"dbgp{j}")
                            nc.vector.tensor_copy(
                                dbgp[:, :],
                                ptall[:, 1024 * j:1024 * j + E])
                            nc.sync.dma_start(
                                out=out[128 * j:128 * j + 128, :],
                                in_=dbgp[:, :])
                    if KSTAGE <= 3:
                        continue

                    # PV + Z accumulation: OT[65, SQ] in two halves, row 64 = Z
                    oth = [otp.tile([128, 512], F32, tag=f"ot{qh}",
                                    name=f"ot{qh}_{h}") for qh in range(2)]
                    for qh in range(2):
                        for kt in range(NKT):
                            nc.tensor.matmul(
                                oth[qh][0:65, :],
                                v_sb[kt][:, 65 * h:65 * h + 65],
                                ptall[:, 1024 * kt + 512 * qh:
                                      1024 * kt + 512 * qh + 512],
                                start=(kt == 0), stop=(kt == NKT - 1))

                    zrow = stats.tile([1, SQ], F32, tag="zrow",
                                      name=f"zrow{h}")
                    for qh in range(2):
                        nc.vector.tensor_scalar(
                            out=zrow[:, 512 * qh:512 * qh + 512],
                            in0=oth[qh][64:65, :], scalar1=1.0e-30,
                            scalar2=None, op0=ALU.max)
                    # reciprocal on 128 partitions (1-partition DVE recip is
                    # ~6 cycles/elem); two tiny DMA hops re/de-interleave.
                    zt = stats.tile([128, 8], F32, tag="zt", name=f"zt{h}")
                    nc.sync.dma_start(out=zt[:, :], in_=zrow[:, :])
                    rzt = stats.tile([128, 8], F32, tag="rzt", name=f"rzt{h}")
                    nc.vector.reciprocal(rzt[:, :], zt[:, :])
                    rz = stats.tile([1, SQ], F32, tag="rz", name=f"rz{h}")
                    nc.sync.dma_start(out=rz[:, :], in_=rzt[:, :])
                    rzbc = rzp.tile([64, SQ], F32, tag="rzbc", name=f"rzbc{h}")
                    nc.gpsimd.partition_broadcast(rzbc[:, :], rz[:, :])

                    # scramble copy with 1/Z folded in:
                    # otd2[64a+d, 128c+t] = OT[d, 128(2c+a)+t] * rz[...]
                    otd2 = odp.tile([128, 512], BF16, tag="otd2",
                                    name=f"otd2{h}")
                    rz_r = rzbc.rearrange("p (j t) -> p j t", j=8)
                    for a in range(2):
                        for ch in range(2):
                            # global block jj=2c+a: c in {2ch, 2ch+1} ->
                            # oth[ch] local blocks {a, a+2}
                            ot_r = oth[ch].rearrange("p (j t) -> p j t", j=4)
                            nc.vector.tensor_tensor(
                                out=otd2[64 * a:64 * a + 64,
                                         256 * ch:256 * ch + 256].rearrange(
                                    "p (c t) -> p c t", c=2),
                                in0=ot_r[0:64, a::2, :],
                                in1=rz_r[0:64,
                                         4 * ch + a:4 * ch + a + 3:2, :],
                                op=ALU.mult)

                    po = otp.tile([128, 512], F32, tag="po", name=f"po{h}")
                    for c in range(4):
                        nc.tensor.matmul(
                            po[:, :], otd2[:, 128 * c:128 * c + 128],
                            woc_sb[c][:, :], start=(c == 0), stop=(c == 3))
                    o_sb = odp.tile([128, E], F32, tag="osb", name=f"osb{h}")
                    nc.vector.tensor_tensor(
                        out=o_sb[:, :], in0=po[:, :], in1=bo_sb[:, :],
                        op=ALU.add)
                    nc.sync.dma_start(out=out[128 * h:128 * h + 128, :],
                                      in_=o_sb[:, :])

    nc.compile()
    return nc


_NC = None
_last_in_maps = None

# j-major query permutation: position p <-> q row 8*(p%128) + p//128
_QPERM = (8 * (np.arange(SQ) % 128) + np.arange(SQ) // 128)

# output-projection row scramble: woc[128c + 64a + d] = WoT row 64*(2c+a)+d
_p = np.arange(E)
_FIDX = 64 * (2 * (_p // 128) + (_p % 128) // 64) + (_p % 64)


def _get_nc():
    global _NC
    if _NC is None:
        _NC = build_nc()
    return _NC


def kernel(query, key_in, value, mask, Wq, bq, Wk, bk, Wv, bv, Wo, bo):
    query = np.asarray(query, np.float32)
    key_in = np.asarray(key_in, np.float32)
    value = np.asarray(value, np.float32)
    mask = np.asarray(mask)
    Wq = np.asarray(Wq, np.float32)
    Wk = np.asarray(Wk, np.float32)
    Wv = np.asarray(Wv, np.float32)
    Wo = np.asarray(Wo, np.float32)

    sdk = np.sqrt(np.float32(DK))
    wqT = np.ascontiguousarray((Wq * sdk).T)
    wkT = np.ascontiguousarray(Wk.T)
    wvT = np.ascontiguousarray(Wv.T).astype(BF)
    woc = np.ascontiguousarray(Wo.T[_FIDX, :]).astype(BF)
    bqt = np.ascontiguousarray(
        (np.asarray(bq, np.float32) * sdk).reshape(NE, 128).T)
    bkt = np.ascontiguousarray(np.asarray(bk, np.float32).reshape(NE, 128).T)
    bvr = np.ascontiguousarray(
        np.tile(np.asarray(bv, np.float32).reshape(1, E), (128, 1)))
    bor = np.ascontiguousarray(
        np.tile(np.asarray(bo, np.float32).reshape(1, E), (128, 1)))

    in_maps = []
    for c in range(N_CORES):
        b, r = c // 2, c % 2
        q0 = SQ * r
        qsel = q0 + _QPERM
        mbias = np.where(mask[b, qsel, :] == 0, np.float32(NEG),
                         np.float32(0.0)).astype(BF)
        in_maps.append({
            "xqT": np.ascontiguousarray(query[b, qsel, :].T),
            "xkT": np.ascontiguousarray(key_in[b].T),
            "xvT": np.ascontiguousarray(value[b].T).astype(BF),
            "mb": mbias,
            "wqT": wqT, "wkT": wkT, "wvT": wvT, "woc": woc,
            "bqt": bqt, "bkt": bkt, "bvr": bvr, "bor": bor,
        })

    nc = _get_nc()
    global _last_in_maps
    _last_in_maps = in_maps
    res = run_bass_kernel_spmd(nc, in_maps, list(range(N_CORES)))

    full = np.empty((B, S, E), np.float32)
    for c in range(N_CORES):
        b, r = c // 2, c % 2
        oc = res.results[c]["out"]
        for h in range(H):
            full[b, 256 * h + 128 * r:256 * h + 128 * r + 128, :] = \
                oc[128 * h:128 * h + 128, :]
    return full


# revision 23
# speedup vs baseline: 1.1583x; 1.0613x over previous
"""Multi-head attention Trainium2 kernel (8 NeuronCores, SPMD).

Sharding: core c handles batch b=c//2, query-row half r=c%2 (1024 q rows),
all 8 heads, full key range.

Per-core pipeline (S-orientation scores, fused softmax):
  QT = (8*Wq^T) x_q, KT = Wk^T x_k  (fp32r), V = x_v Wv^T (bf16, with a
  ones column appended per head so PV also accumulates Z = row-sums).
  per (head, 128-row q tile):
    S = QT_h^T KT_h                  (PSUM fp32, q on partitions)
    one fused DVE pass: msc = -(S + maskbias), nmx = min(msc) = -rowmax
    P = exp(-msc + nmx)              (ACT, bf16; masked entries -> 0)
    PT strips via DMA crossbar transpose (sbuf->sbuf, 16x128 xbar tiles)
  per head: OT[65,1024] = V'^T PT accumulated over k (row 64 = Z), then
  1/Z broadcast (Pool partition_broadcast) is folded into the scramble
  copy, and the reference's head-scrambled reshape is folded into the
  output projection as 4 K=128 matmuls against pre-scrambled Wo chunks.
Query rows are processed in j-major permuted order (q = 8t+j <-> p =
t+128j) so every access stays contiguous; the host permutes query/mask
rows and reassembles the output.
"""

import os
import numpy as np
import ml_dtypes

KSTAGE = int(os.environ.get("KSTAGE", "9"))

import concourse.bass as bass
import concourse.mybir as mybir
from concourse import bacc
from concourse.bass_utils import run_bass_kernel_spmd
from concourse.tile import TileContext
from concourse.masks import make_identity

F32 = mybir.dt.float32
F32R = mybir.dt.float32r
BF16 = mybir.dt.bfloat16
AF = mybir.ActivationFunctionType
ALU = mybir.AluOpType

B, S, E, H, DK = 4, 2048, 512, 8, 64
SQ = S // 2          # q rows per core
NE = E // 128        # 4 embed chunks
NKT = S // 128       # 16 key tiles
NQT = SQ // 128      # 8 q tiles per core
NEG = -1.0e9
N_CORES = 8

BF = ml_dtypes.bfloat16


def build_nc():
    nc = bacc.Bacc(None, target_bir_lowering=False)

    xqT = nc.declare_dram_parameter("xqT", [E, SQ], F32R, isOutput=False)
    xkT = nc.declare_dram_parameter("xkT", [E, S], F32R, isOutput=False)
    xvT = nc.declare_dram_parameter("xvT", [E, S], BF16, isOutput=False)
    mb = nc.declare_dram_parameter("mb", [SQ, S], BF16, isOutput=False)
    wqT = nc.declare_dram_parameter("wqT", [E, E], F32R, isOutput=False)
    wkT = nc.declare_dram_parameter("wkT", [E, E], F32R, isOutput=False)
    wvT = nc.declare_dram_parameter("wvT", [E, E], BF16, isOutput=False)
    woc = nc.declare_dram_parameter("woc", [E, E], BF16, isOutput=False)
    bqt = nc.declare_dram_parameter("bqt", [128, NE], F32, isOutput=False)
    bkt = nc.declare_dram_parameter("bkt", [128, NE], F32, isOutput=False)
    bvr = nc.declare_dram_parameter("bvr", [128, E], F32, isOutput=False)
    bor = nc.declare_dram_parameter("bor", [128, E], F32, isOutput=False)
    out = nc.declare_dram_parameter("out", [SQ, E], F32, isOutput=True)

    with TileContext(nc) as tc:
        with (
            tc.tile_pool(name="const", bufs=1) as constp,
            tc.tile_pool(name="mbp", bufs=1) as mbp,
            tc.tile_pool(name="qk", bufs=1) as qk,
            tc.tile_pool(name="vp", bufs=1) as vp,
        ):
            bq_sb = constp.tile([128, NE], F32, tag="bq", name="bq")
            bk_sb = constp.tile([128, NE], F32, tag="bk", name="bk")
            bv_sb = constp.tile([128, E], F32, tag="bv", name="bv")
            bo_sb = constp.tile([128, E], F32, tag="bo", name="bo")
            nc.sync.dma_start(out=bq_sb[:, :], in_=bqt[:, :])
            nc.sync.dma_start(out=bk_sb[:, :], in_=bkt[:, :])
            nc.sync.dma_start(out=bv_sb[:, :], in_=bvr[:, :])
            nc.sync.dma_start(out=bo_sb[:, :], in_=bor[:, :])
            woc_sb = [constp.tile([128, E], BF16, tag=f"woc{c}", name=f"woc{c}")
                      for c in range(4)]
            for c in range(4):
                nc.sync.dma_start(out=woc_sb[c][:, :],
                                  in_=woc[128 * c:128 * c + 128, :])

            mb_sb = [mbp.tile([128, S], BF16, tag=f"mb{j}", name=f"mb{j}")
                     for j in range(NQT)]
            for j in range(NQT):
                nc.sync.dma_start(out=mb_sb[j][:, :],
                                  in_=mb[128 * j:128 * j + 128, :])

            qt_sb = [qk.tile([128, SQ], F32R, tag=f"qt{m}", name=f"qt{m}")
                     for m in range(NE)]
            kt_sb = [qk.tile([128, S], F32R, tag=f"kt{m}", name=f"kt{m}")
                     for m in range(NE)]
            v_sb = [vp.tile([128, 8 * 65], BF16, tag=f"v{k}", name=f"v{k}")
                    for k in range(NKT)]
            for k in range(NKT):
                ones_ap = v_sb[k].rearrange("p (h d) -> p h d", h=H)[:, :, 64:65]
                nc.gpsimd.memset(ones_ap, 1.0)

            # ---------------- projection phase ----------------
            with (
                tc.tile_pool(name="wts", bufs=1) as wts,
                tc.tile_pool(name="xqp", bufs=1) as xqp,
                tc.tile_pool(name="xkp", bufs=2) as xkp,
                tc.tile_pool(name="xvp", bufs=2) as xvp,
                tc.tile_pool(name="pps", bufs=2, space="PSUM") as pps,
            ):
                wq_sb = [wts.tile([128, E], F32R, tag=f"wq{c}", name=f"wq{c}")
                         for c in range(NE)]
                wk_sb = [wts.tile([128, E], F32R, tag=f"wk{c}", name=f"wk{c}")
                         for c in range(NE)]
                wv_sb = [wts.tile([128, E], BF16, tag=f"wv{c}", name=f"wv{c}")
                         for c in range(NE)]
                xq_sb = [xqp.tile([128, SQ], F32R, tag=f"xq{c}", name=f"xq{c}")
                         for c in range(NE)]
                for c in range(NE):
                    nc.sync.dma_start(out=wq_sb[c][:, :],
                                      in_=wqT[128 * c:128 * c + 128, :])
                    nc.sync.dma_start(out=wk_sb[c][:, :],
                                      in_=wkT[128 * c:128 * c + 128, :])
                    nc.sync.dma_start(out=wv_sb[c][:, :],
                                      in_=wvT[128 * c:128 * c + 128, :])
                    nc.sync.dma_start(out=xq_sb[c][:, :],
                                      in_=xqT[128 * c:128 * c + 128, :])

                # Q projection: QT[e',q'] chunks
                for m in range(NE):
                    ps = pps.tile([128, 1024], F32, tag="pps", name=f"psq{m}")
                    for h2 in range(2):
                        for c in range(NE):
                            nc.tensor.matmul(
                                ps[:, 512 * h2:512 * h2 + 512],
                                wq_sb[c][:, 128 * m:128 * m + 128],
                                xq_sb[c][:, 512 * h2:512 * h2 + 512],
                                start=(c == 0), stop=(c == NE - 1))
                    nc.scalar.activation(
                        out=qt_sb[m][:, :], in_=ps[:, :], func=AF.Identity,
                        bias=bq_sb[:, m:m + 1], scale=1.0)

                # K projection, x_k staged in halves
                for kh in range(2):
                    xk_sb = [xkp.tile([128, 1024], F32R, tag=f"xk{c}",
                                      name=f"xk{c}_{kh}") for c in range(NE)]
                    for c in range(NE):
                        nc.sync.dma_start(
                            out=xk_sb[c][:, :],
                            in_=xkT[128 * c:128 * c + 128,
                                    1024 * kh:1024 * kh + 1024])
                    for m in range(NE):
                        ps = pps.tile([128, 1024], F32, tag="pps",
                                      name=f"psk{m}_{kh}")
                        for kq in range(2):
                            for c in range(NE):
                                nc.tensor.matmul(
                                    ps[:, 512 * kq:512 * kq + 512],
                                    wk_sb[c][:, 128 * m:128 * m + 128],
                                    xk_sb[c][:, 512 * kq:512 * kq + 512],
                                    start=(c == 0), stop=(c == NE - 1))
                        nc.scalar.activation(
                            out=kt_sb[m][:, 1024 * kh:1024 * kh + 1024],
                            in_=ps[:, :], func=AF.Identity,
                            bias=bk_sb[:, m:m + 1], scale=1.0)

                # V projection, x_v staged in halves; V' = [V | 1] per head
                for vh in range(2):
                    xv_sb = [xvp.tile([128, 1024], BF16, tag=f"xv{c}",
                                      name=f"xv{c}_{vh}") for c in range(NE)]
                    for c in range(NE):
                        nc.sync.dma_start(
                            out=xv_sb[c][:, :],
                            in_=xvT[128 * c:128 * c + 128,
                                    1024 * vh:1024 * vh + 1024])
                    for ppp in range(4):
                        ps = pps.tile([128, 1024], F32, tag="pps",
                                      name=f"psv{ppp}_{vh}")
                        for sub in range(2):
                            kl = 256 * ppp + 128 * sub  # local k col in half
                            for c in range(NE):
                                nc.tensor.matmul(
                                    ps[:, 512 * sub:512 * sub + 512],
                                    xv_sb[c][:, kl:kl + 128],
                                    wv_sb[c][:, :],
                                    start=(c == 0), stop=(c == NE - 1))
                        for sub in range(2):
                            kt = 8 * vh + 2 * ppp + sub
                            vout = v_sb[kt].rearrange(
                                "p (h d) -> p h d", h=H)[:, :, 0:64]
                            vin = ps[:, 512 * sub:512 * sub + 512].rearrange(
                                "p (h d) -> p h d", h=H)
                            bvv = bv_sb.rearrange("p (h d) -> p h d", h=H)
                            nc.vector.tensor_tensor(
                                out=vout, in0=vin[:, :, :], in1=bvv[:, :, :],
                                op=ALU.add)

            # ---------------- attention phase ----------------
            with (
                tc.tile_pool(name="ptp", bufs=2) as ptp,
                tc.tile_pool(name="pp", bufs=2) as pp,
                tc.tile_pool(name="stats", bufs=2) as stats,
                tc.tile_pool(name="rzp", bufs=2) as rzp,
                tc.tile_pool(name="odp", bufs=2) as odp,
                tc.tile_pool(name="idp", bufs=1) as idp,
                tc.tile_pool(name="scp", bufs=5, space="PSUM") as scp,
                tc.tile_pool(name="otp", bufs=1, space="PSUM") as otp,
            ):
                id_bf = idp.tile([128, 128], BF16, tag="id_bf", name="id_bf")
                make_identity(nc, id_bf[:, :])
                if KSTAGE <= 1:
                    # proj debug: dump QT chunk 0
                    dbg = odp.tile([128, E], F32, tag="dbg", name="dbg")
                    nc.vector.tensor_copy(dbg[:, :],
                                          qt_sb[0][:, 0:E].bitcast(F32))
                    nc.sync.dma_start(out=out[0:128, :], in_=dbg[:, :])
                ptalls = {}
                oths = {}

                def scores_block(h, j):
                    hm, hp = h // 2, 64 * (h % 2)
                    qch = qt_sb[hm][hp:hp + 64, 128 * j:128 * j + 128]
                    nmx = stats.tile([128, 6], F32, tag="nmx",
                                     name=f"nmx{h}_{j}")
                    scq = []
                    for kq in range(4):
                        sc = scp.tile([128, 512], F32, tag="sc",
                                      name=f"sc{h}_{j}_{kq}")
                        scq.append(sc)
                        ksl = slice(512 * kq, 512 * kq + 512)
                        nc.tensor.matmul(
                            sc[:, :], qch, kt_sb[hm][hp:hp + 64, ksl],
                            start=True, stop=False)
                        nc.tensor.matmul(
                            sc[:, :], id_bf[:, :], mb_sb[j][:, ksl],
                            start=False, stop=True)
                        nc.vector.tensor_reduce(
                            out=nmx[:, kq:kq + 1], in_=sc[:, :],
                            axis=mybir.AxisListType.X, op=ALU.max)
                    nc.vector.tensor_reduce(
                        out=nmx[:, 4:5], in_=nmx[:, 0:4],
                        axis=mybir.AxisListType.X, op=ALU.max, negate=True)
                    p_sb = pp.tile([128, S], BF16, tag="p", name=f"p{h}_{j}")
                    for kq in range(4):
                        nc.scalar.activation(
                            out=p_sb[:, 512 * kq:512 * kq + 512],
                            in_=scq[kq][:, :],
                            func=AF.Exp, bias=nmx[:, 4:5], scale=1.0)
                    ptall_r = ptalls[h].rearrange("p (s q) -> p s q", s=NKT)
                    xeng = nc.sync if j % 2 == 0 else nc.scalar
                    xeng.dma_start_transpose(
                        ptall_r[:, :, 128 * j:128 * j + 128], p_sb[:, :])

                def pv_chunk(h, qh, k0):
                    # PV + Z: OT[65, SQ] in two halves, row 64 = Z.
                    # q-half qh only needs transposes j in [4qh, 4qh+4).
                    if qh == 0 and k0 == 0:
                        oths[h] = [otp.tile([128, 512], F32, tag=f"ot{q2}",
                                            name=f"ot{q2}_{h}")
                                   for q2 in range(2)]
                    for kt in range(k0, k0 + 8):
                        nc.tensor.matmul(
                            oths[h][qh][0:65, :],
                            v_sb[kt][:, 65 * h:65 * h + 65],
                            ptalls[h][:, 1024 * kt + 512 * qh:
                                      1024 * kt + 512 * qh + 512],
                            start=(kt == 0), stop=(kt == NKT - 1))

                def tail(h):
                    oth = oths.pop(h)
                    zrow = stats.tile([1, SQ], F32, tag="zrow",
                                      name=f"zrow{h}")
                    for qh in range(2):
                        nc.vector.tensor_scalar(
                            out=zrow[:, 512 * qh:512 * qh + 512],
                            in0=oth[qh][64:65, :], scalar1=1.0e-30,
                            scalar2=None, op0=ALU.max)
                    # reciprocal on 128 partitions (1-partition DVE recip
                    # is ~6 cycles/elem); two tiny DMA hops re/de-interleave
                    zt = stats.tile([128, 8], F32, tag="zt", name=f"zt{h}")
                    nc.sync.dma_start(out=zt[:, :], in_=zrow[:, :])
                    rzt = stats.tile([128, 8], F32, tag="rzt",
                                     name=f"rzt{h}")
                    nc.vector.reciprocal(rzt[:, :], zt[:, :])
                    rz = stats.tile([1, SQ], F32, tag="rz", name=f"rz{h}")
                    nc.sync.dma_start(out=rz[:, :], in_=rzt[:, :])
                    rzbc = rzp.tile([64, SQ], F32, tag="rzbc",
                                    name=f"rzbc{h}")
                    nc.gpsimd.partition_broadcast(rzbc[:, :], rz[:, :])

                    # scramble copy with 1/Z folded in:
                    # otd2[64a+d, 128c+t] = OT[d, 128(2c+a)+t] * rz[...]
                    otd2 = odp.tile([128, 512], BF16, tag="otd2",
                                    name=f"otd2{h}")
                    rz_r = rzbc.rearrange("p (j t) -> p j t", j=8)
                    for a in range(2):
                        for ch in range(2):
                            # global block jj=2c+a: c in {2ch, 2ch+1} ->
                            # oth[ch] local blocks {a, a+2}
                            ot_r = oth[ch].rearrange("p (j t) -> p j t", j=4)
                            nc.vector.tensor_tensor(
                                out=otd2[64 * a:64 * a + 64,
                                         256 * ch:256 * ch + 256].rearrange(
                                    "p (c t) -> p c t", c=2),
                                in0=ot_r[0:64, a::2, :],
                                in1=rz_r[0:64,
                                         4 * ch + a:4 * ch + a + 3:2, :],
                                op=ALU.mult)

                    po = otp.tile([128, 512], F32, tag="po", name=f"po{h}")
                    for c in range(4):
                        nc.tensor.matmul(
                            po[:, :], otd2[:, 128 * c:128 * c + 128],
                            woc_sb[c][:, :], start=(c == 0), stop=(c == 3))
                    o_sb = odp.tile([128, E], F32, tag="osb", name=f"osb{h}")
                    nc.vector.tensor_tensor(
                        out=o_sb[:, :], in0=po[:, :], in1=bo_sb[:, :],
                        op=ALU.add)
                    nc.sync.dma_start(out=out[128 * h:128 * h + 128, :],
                                      in_=o_sb[:, :])

                # software pipeline: head h's scores interleave with head
                # h-1's PV chunks and tail, so PE always has fillable work
                NH = H if KSTAGE >= 4 else (0 if KSTAGE <= 1 else 1)
                for hh in range(NH + 1):
                    if hh < NH:
                        ptalls[hh] = ptp.tile([128, NKT * 1024], BF16,
                                              tag="ptall",
                                              name=f"ptall{hh}")
                    for j in range(NQT):
                        if hh < NH:
                            scores_block(hh, j)
                        if hh >= 1 and KSTAGE >= 4:
                            hv = hh - 1
                            if j == 0:
                                pv_chunk(hv, 0, 0)
                            elif j == 1:
                                pv_chunk(hv, 0, 8)
                            elif j == 2:
                                pv_chunk(hv, 1, 0)
                            elif j == 3:
                                pv_chunk(hv, 1, 8)
                            elif j == 4:
                                tail(hv)
                                ptalls.pop(hv)

    nc.compile()
    return nc


_NC = None
_last_in_maps = None

# j-major query permutation: position p <-> q row 8*(p%128) + p//128
_QPERM = (8 * (np.arange(SQ) % 128) + np.arange(SQ) // 128)

# output-projection row scramble: woc[128c + 64a + d] = WoT row 64*(2c+a)+d
_p = np.arange(E)
_FIDX = 64 * (2 * (_p // 128) + (_p % 128) // 64) + (_p % 64)


def _get_nc():
    global _NC
    if _NC is None:
        _NC = build_nc()
    return _NC


def kernel(query, key_in, value, mask, Wq, bq, Wk, bk, Wv, bv, Wo, bo):
    query = np.asarray(query, np.float32)
    key_in = np.asarray(key_in, np.float32)
    value = np.asarray(value, np.float32)
    mask = np.asarray(mask)
    Wq = np.asarray(Wq, np.float32)
    Wk = np.asarray(Wk, np.float32)
    Wv = np.asarray(Wv, np.float32)
    Wo = np.asarray(Wo, np.float32)

    sdk = np.sqrt(np.float32(DK))
    wqT = np.ascontiguousarray((Wq * sdk).T)
    wkT = np.ascontiguousarray(Wk.T)
    wvT = np.ascontiguousarray(Wv.T).astype(BF)
    woc = np.ascontiguousarray(Wo.T[_FIDX, :]).astype(BF)
    bqt = np.ascontiguousarray(
        (np.asarray(bq, np.float32) * sdk).reshape(NE, 128).T)
    bkt = np.ascontiguousarray(np.asarray(bk, np.float32).reshape(NE, 128).T)
    bvr = np.ascontiguousarray(
        np.tile(np.asarray(bv, np.float32).reshape(1, E), (128, 1)))
    bor = np.ascontiguousarray(
        np.tile(np.asarray(bo, np.float32).reshape(1, E), (128, 1)))

    in_maps = []
    for c in range(N_CORES):
        b, r = c // 2, c % 2
        q0 = SQ * r
        qsel = q0 + _QPERM
        mbias = np.where(mask[b, qsel, :] == 0, np.float32(NEG),
                         np.float32(0.0)).astype(BF)
        in_maps.append({
            "xqT": np.ascontiguousarray(query[b, qsel, :].T),
            "xkT": np.ascontiguousarray(key_in[b].T),
            "xvT": np.ascontiguousarray(value[b].T).astype(BF),
            "mb": mbias,
            "wqT": wqT, "wkT": wkT, "wvT": wvT, "woc": woc,
            "bqt": bqt, "bkt": bkt, "bvr": bvr, "bor": bor,
        })

    nc = _get_nc()
    global _last_in_maps
    _last_in_maps = in_maps
    res = run_bass_kernel_spmd(nc, in_maps, list(range(N_CORES)))

    full = np.empty((B, S, E), np.float32)
    for c in range(N_CORES):
        b, r = c // 2, c % 2
        oc = res.results[c]["out"]
        for h in range(H):
            full[b, 256 * h + 128 * r:256 * h + 128 * r + 128, :] = \
                oc[128 * h:128 * h + 128, :]
    return full
